# revision 38
# baseline (speedup 1.0000x reference)
"""GQA attention block (B=2, N=2048, D=2048, H=16, HKV=4, HD=128) on 8 TRN2 cores.

Sharding: core c -> batch b = c // 4, query-row quarter j = c % 4 (512 rows).

v2 schedule (HAM-sustained PE clock is 13/16 * 2.4GHz = 1.946 GHz, so the
PE-stream floor is ~304us/core; every phase below is packed against it):
  - K+V projection fused, dc-outer: 8 PSUM tiles (4 K heads d-major + 4 V
    n-blocks) accumulate per contraction chunk, so the first matmul needs
    only ~1.5MB of DMA (xts chunk 0 + wk + wv chunk 0) and compute starts
    ~15us in.  RoPE(K) + bounce-out after the dc loop.
  - ALL input DMAs (including the full 8MB Wq) are emitted before the
    bounce writes: the AllGather readiness gate covers them, and the AG
    data phase then runs with zero concurrent-DMA starvation while the
    (DMA-free) Q projection streams on the PE.
  - The remote K/V gather reads for the first two kv heads are emitted
    right after the collective, so they fire the moment the AG completes
    and the attention stream never waits on remote keys (v1 lost ~10us +
    a HAM clock drop to 8/16 at the Q->attention transition).
  - attention in transposed-score form: S^T = K.Q^T, exp on ScalarE (the
    binding engine at ~128us: 131072 elems/partition at 1.2GHz), keys in
    one 16-block accumulation per head grouped in triples per exp op.
    Denominator via ones-column appended to V.  Per-head PV PSUM banks are
    freed by a fast raw copy (DVE); reciprocal+normalize+transpose run
    off the inter-head critical chain.
  - output projection over all heads -> 512 output rows, bf16 output DMA
    (host widens to f32 and adds bo).
All matmuls bf16 with f32 PSUM accumulation; softmax statistics in f32.
A burst of zero-weight matmuls at kernel start warms the PE HAM clock gate
(cold PE runs at 1.2 GHz) while the first input DMAs are still in flight.
"""

import numpy as np
import ml_dtypes

import concourse.bass as bass
import concourse.mybir as mybir
import concourse.tile as tile
from concourse import bacc
from concourse.bass_utils import run_bass_kernel_spmd

B, N, D = 2, 2048, 2048
H, HKV, HD = 16, 4, 128
G = H // HKV
NQ = N // 4          # query rows per core
DC = D // 128        # contraction chunks for projections
KB = N // 128        # key blocks per batch
NCORES = 8
SCALE = float(HD) ** -0.5

BF16 = mybir.dt.bfloat16
F32 = mybir.dt.float32
MUL = mybir.AluOpType.mult
ADD = mybir.AluOpType.add
EXP = mybir.ActivationFunctionType.Exp

_cache = {}


def _rope(nc, pool, out, in_psum, cos2_sb, sin2_sb):
    """Rotate-half RoPE with head-dim on partitions.

    cos2_sb = [cos; cos], sin2_sb = [sin; -sin] (128 rows, host-prepared), so
    out = t*cos2 + rot(t)*sin2 where rot swaps the partition halves.
    ScalarE (idle during projections) does the PSUM reads; the three DVE
    multiplies/adds then run all-SBUF at the 2x f32 rate.
    """
    rot = pool.tile([128, NQ], F32, name="rope_rot")
    nc.scalar.copy(rot[0:64, :], in_psum[64:128, :])
    nc.scalar.copy(rot[64:128, :], in_psum[0:64, :])
    m1 = pool.tile([128, NQ], F32, name="rope_m1")
    m2 = pool.tile([128, NQ], F32, name="rope_m2")
    nc.vector.tensor_tensor(m1[:], in_psum[:], cos2_sb[:], MUL)
    nc.vector.tensor_tensor(m2[:], rot[:], sin2_sb[:], MUL)
    nc.vector.tensor_tensor(out[:], m1[:], m2[:], ADD)


def _build():
    from contextlib import ExitStack

    nc = bacc.Bacc("TRN2", target_bir_lowering=False, debug=False,
                   num_devices=NCORES)

    xT_d = nc.dram_tensor("xT", [D, NQ], BF16, kind="ExternalInput").ap()
    cosT_d = nc.dram_tensor("cosT", [HD, NQ], F32, kind="ExternalInput").ap()
    sinT_d = nc.dram_tensor("sinT", [HD, NQ], F32, kind="ExternalInput").ap()
    wq_d = nc.dram_tensor("wq", [H, 128, DC, 128], BF16, kind="ExternalInput").ap()
    wk_d = nc.dram_tensor("wk", [HKV, 128, DC, 128], BF16, kind="ExternalInput").ap()
    wv_d = nc.dram_tensor("wv", [DC, 128, HKV * HD], BF16, kind="ExternalInput").ap()
    wo_d = nc.dram_tensor("wo", [H, 128, D], BF16, kind="ExternalInput").ap()
    out_d = nc.dram_tensor("out", [NQ, D], BF16, kind="ExternalOutput").ap()

    with tile.TileContext(nc) as tc, ExitStack() as top:
        resident = top.enter_context(tc.tile_pool(name="resident", bufs=1))
        dram = top.enter_context(tc.tile_pool(name="dram", bufs=1, space="DRAM"))
        # remote K/V tiles outlive the projection scope -> enter before it;
        # bufs=4 holds every kv head at once so no gather read is ever
        # WAR-blocked at the head of the DMA ring (damming traffic behind it)
        ktpool = top.enter_context(tc.tile_pool(name="ktpool", bufs=4))
        vppool = top.enter_context(tc.tile_pool(name="vppool", bufs=4))

        warmw = resident.tile([128, 128], BF16)
        nc.gpsimd.memset(warmw[:], 0.0)

        # HAM warm-up: the PE clock gate defaults to 1.2 GHz and needs ~3.4us
        # of sustained matmul activity to lift.  These zero-weight matmuls
        # depend on no DMA, so they run while the first input chunks are
        # still in flight and the real matmul stream starts warm.
        with tc.tile_pool(name="warm", bufs=1, space="PSUM") as wp:
            wps = wp.tile([128, 128], F32)
            for _ in range(64):
                nc.tensor.matmul(wps[:], warmw[:], warmw[:],
                                 start=True, stop=True)

        q_sb = [resident.tile([128, NQ], BF16, name=f"q_sb{_h}")
                for _h in range(H)]
        k_loc = resident.tile([128, HKV, NQ], BF16)       # roped local K, d-major
        vp_loc = resident.tile([128, HKV * 4, HD + 1], BF16)  # (hk, local kb)
        nc.gpsimd.memset(vp_loc[:, :, HD:HD + 1], 1.0)
        oT_sb = [resident.tile([128, 4, 128], BF16, name=f"oT{_h}")
                 for _h in range(H)]

        # kv bounce: rows 0..511 = roped K (4 heads x 128 d), cols = local n;
        # rows 512..1023 = V (local n rows), cols = 4 heads x 128 channels
        kv_bounce = dram.tile([2 * NQ, NQ], BF16)
        ag_out = dram.tile([2 * NQ * 4, NQ], BF16)

        # -- projection scope: tensors freed after the Q phase ------------------
        proj_scope = ExitStack()
        proj = proj_scope.enter_context(tc.tile_pool(name="proj", bufs=1))
        tmp_pool = proj_scope.enter_context(tc.tile_pool(name="ropetmp", bufs=3))
        cos_sb = proj.tile([HD, NQ], F32)
        sin_sb = proj.tile([HD, NQ], F32)
        xts = proj.tile([128, DC, NQ], BF16)
        xT_r = xT_d.rearrange("(dc p) n -> p dc n", p=128)
        wq_sb = proj.tile([128, H, DC * 128], BF16)

        # ---------------- KV projection + RoPE(K) + bounce-out ----------------
        with ExitStack() as ph:
            wkpool = ph.enter_context(tc.tile_pool(name="wkpool", bufs=1))
            wvpool = ph.enter_context(tc.tile_pool(name="wvpool", bufs=1))
            kvsb = ph.enter_context(tc.tile_pool(name="kvsb", bufs=3))
            # PSUM entry order fixes bank addresses: kvps 0-3, vsp 4,
            # qps 5-7.  The attention pools then overlap as stA 0-2 (K
            # banks, drained early), ops 3-4 (K/V banks), stB 5-7 (the qps
            # banks, whose pool-coarse WAR on the last Q RoPE reads lands
            # on the SECOND score chunk, ~1.6us late by construction).
            kvps_pool = proj_scope.enter_context(
                tc.tile_pool(name="kvps", bufs=4, space="PSUM"))
            vsp_pool = proj_scope.enter_context(
                tc.tile_pool(name="vsp", bufs=1, space="PSUM"))
            qps_pool = proj_scope.enter_context(
                tc.tile_pool(name="qps", bufs=1, space="PSUM"))

            wk_sb = wkpool.tile([128, HKV, DC, 128], BF16)
            wv_sb = wvpool.tile([128, DC, HKV * HD], BF16)
            wq_r = wq_d.rearrange("h p dc c -> p h (dc c)")
            wk_r = wk_d.rearrange("hk p dc c -> p hk (dc c)")

            # DMA issue order == compute consumption order (the ring drains
            # in order).  Fine chunks up front so the first K matmul needs
            # only ~2.5MB; wq groups 0-1 ride the early stream (Q proj
            # starts the moment the KV phase drains); wq groups 2-3 are
            # emitted after the bounce writes, excluded from the AllGather's
            # position-based readiness gate, and stream inside its trigger
            # delay, clear of the AG data phase.
            nc.sync.dma_start(wk_sb[:, 0:1, :, :], wk_r[:, 0:1, :])
            nc.sync.dma_start(xts[:, 0:4, :], xT_r[:, 0:4, :])
            nc.sync.dma_start(xts[:, 4:8, :], xT_r[:, 4:8, :])
            nc.sync.dma_start(xts[:, 8:12, :], xT_r[:, 8:12, :])
            nc.sync.dma_start(xts[:, 12:16, :], xT_r[:, 12:16, :])
            nc.sync.dma_start(wk_sb[:, 1:4, :, :], wk_r[:, 1:4, :])
            nc.sync.dma_start(wv_sb[:], wv_d.rearrange("d p c -> p d c"))
            nc.sync.dma_start(cos_sb[:], cosT_d)
            nc.sync.dma_start(sin_sb[:], sinT_d)
            nc.sync.dma_start(wq_sb[:, 0:4, :], wq_r[:, 0:4, :])
            nc.sync.dma_start(wq_sb[:, 4:8, :], wq_r[:, 4:8, :])

            for hk in range(HKV):
                kps = kvps_pool.tile([128, NQ], F32, name="kvps_t")
                for dc in range(DC):
                    nc.tensor.matmul(kps[:], wk_sb[:, hk, dc, :], xts[:, dc, :],
                                     start=(dc == 0), stop=(dc == DC - 1))
                _rope(nc, tmp_pool, k_loc[:, hk, :], kps, cos_sb, sin_sb)
                # scalar-ring DMA: jumps ahead of bulk traffic on the sync
                # ring so the AllGather can start as soon as K/V are ready
                nc.scalar.dma_start(kv_bounce[hk * 128:(hk + 1) * 128, :],
                                    k_loc[:, hk, :])

            # n4-outer with the first tile on the spare PSUM bank: the first
            # V chain starts with no WAR at all, and by the time chain n4=1
            # wants a kvps bank the K RoPE reads (pool-coarse WAR) have
            # drained behind chain n4=0's matmuls.
            vps_tiles = [vsp_pool.tile([128, HKV * HD], F32, name="vsp_t")]
            vps_tiles += [kvps_pool.tile([128, HKV * HD], F32, name="kvps_t")
                          for i in range(3)]
            for n4 in range(4):
                for dc in range(DC):
                    nc.tensor.matmul(
                        vps_tiles[n4][:],
                        xts[:, dc, n4 * 128:(n4 + 1) * 128],
                        wv_sb[:, dc, :],
                        start=(dc == 0), stop=(dc == DC - 1))
            for n4 in range(4):
                v_sb = kvsb.tile([128, HKV * HD], BF16, name="v_sb")
                nc.vector.tensor_copy(v_sb[:], vps_tiles[n4][:])
                nc.scalar.dma_start(
                    kv_bounce[NQ + n4 * 128:NQ + (n4 + 1) * 128, :], v_sb[:])
                for hk in range(HKV):
                    nc.vector.tensor_copy(
                        vp_loc[:, hk * 4 + n4, 0:HD],
                        vps_tiles[n4][:, hk * HD:(hk + 1) * HD])

            # wq groups 2-3 emitted AFTER the bounce DMAs: excluded from the
            # collective's readiness gate, streamed during its trigger delay
            for q4 in range(2, 4):
                nc.sync.dma_start(wq_sb[:, q4 * 4:(q4 + 1) * 4, :],
                                  wq_r[:, q4 * 4:(q4 + 1) * 4, :])

        # ---------------- AllGather K,V within the batch group ----------------
        # Single fused collective (split collectives serialize on the CC core).
        nc.gpsimd.collective_compute(
            "AllGather", mybir.AluOpType.bypass,
            replica_groups=[[0, 1, 2, 3], [4, 5, 6, 7]],
            ins=[kv_bounce.opt()],
            outs=[ag_out.opt()],
        )

        # All four kv-heads' gather reads are emitted NOW so the DMAs fire
        # the moment the collective lands (during Q projection) instead of
        # at attention start.
        pid = nc.sync.partition_id()
        slots = [(pid + i) % 4 for i in (1, 2, 3)]
        hk_tiles = {}

        def load_kv_tiles(hk):
            ktr = ktpool.tile([128, 3, NQ], BF16, name="ktr")
            for i, slot in enumerate(slots):
                nc.sync.dma_start(
                    ktr[:, i, :],
                    ag_out[bass.ds(slot * 2 * NQ + hk * 128, 128), :])
            vpr = vppool.tile([128, 12, HD + 1], BF16, name="vpr")
            nc.gpsimd.memset(vpr[:, :, HD:HD + 1], 1.0)
            for i, slot in enumerate(slots):
                src = ag_out[bass.ds(slot * 2 * NQ + NQ, NQ),
                             hk * HD:(hk + 1) * HD]
                nc.sync.dma_start(
                    vpr[:, i * 4:(i + 1) * 4, 0:HD],
                    src.rearrange("(kbl p) c -> p kbl c", p=128))
            hk_tiles[hk] = (ktr, vpr)

        for _hk in range(HKV):
            load_kv_tiles(_hk)

        # ---------------- Q projection + RoPE ---------------------------------
        # Manual qps rotation (15-h)%3: bank 2 (first st bank the attention
        # scores need) is last touched by h13's RoPE, banks 0-1 (the ops
        # banks, needed a few us later) by h15/h14 -> no attention-start WAR.
        qts = [qps_pool.tile([128, NQ], F32, name=f"qps_t{i}")
               for i in range(3)]
        for h in range(H):
            qps = qts[(15 - h) % 3]
            for dc in range(DC):
                nc.tensor.matmul(qps[:],
                                 wq_sb[:, h, dc * 128:(dc + 1) * 128],
                                 xts[:, dc, :],
                                 start=(dc == 0), stop=(dc == DC - 1))
            _rope(nc, tmp_pool, q_sb[h][:], qps, cos_sb, sin_sb)
        proj_scope.close()

        # Wo lands in the space freed by the projection tensors; its DMAs are
        # emitted inside the attention loop (per kv head) so they queue on
        # the ring behind the AG-gated gather reads and never contend with
        # the collective.
        post = top.enter_context(tc.tile_pool(name="post", bufs=1))
        wo_sb = post.tile([128, H, D], BF16)

        # ---------------- Attention (flat pipelined stream) -------------------
        # One uniform stream of (head, key-block) pairs in triples: each exp
        # op spans head boundaries, so ScalarE (the attention-phase floor)
        # runs continuously and head flushes never stall the PE.
        # PV accumulates 16 blocks per head into two packed 2-slot PSUM tiles
        # (start=True only opens the first slot's chain: it clears has_written
        # for the whole bank, so the second chain opens with start=False on
        # freshly-cleared bits).  Each head's PSUM banks are released by a
        # short raw DVE copy; reciprocal + normalize + the [128,128] XBAR
        # transposes run afterwards, off the inter-head chain.
        # The stream runs in TWO passes to absorb peer-core launch skew (the
        # AllGather's data phase ends only after the slowest peer in the
        # replica group triggers, which can be 30-45us after this core):
        # pass 1 covers every head's LOCAL key blocks (kb 0-3, no collective
        # dependency, ~37us of PE work), spilling each head's partial
        # numerator/denominator to SBUF; pass 2 streams the remote blocks
        # and merges the partials during normalization.
        with ExitStack() as ph:
            ptpool = ph.enter_context(tc.tile_pool(name="ptpool", bufs=5))
            npool = ph.enter_context(tc.tile_pool(name="npool", bufs=4))
            orpool = ph.enter_context(tc.tile_pool(name="orpool", bufs=3))
            opool = ph.enter_context(tc.tile_pool(name="opool", bufs=6))
            lppool = ph.enter_context(tc.tile_pool(name="lppool", bufs=1))
            # all 16 heads' local partials live until pass 2 -> one static tile
            lpart_sb = lppool.tile([128, H, 4, HD + 1], BF16)
            stA_pool = ph.enter_context(tc.tile_pool(name="stA", bufs=1, space="PSUM"))
            ops_pool = ph.enter_context(tc.tile_pool(name="opsp", bufs=1, space="PSUM"))
            stB_pool = ph.enter_context(tc.tile_pool(name="stB", bufs=1, space="PSUM"))
            stt = [stA_pool.tile([128, 3, NQ], F32, name="stA0"),
                   stB_pool.tile([128, 3, NQ], F32, name="stB0")]

            heads = [(hk, g * HKV + hk) for hk in range(HKV) for g in range(G)]

            def kchunk(hk, kb):
                if kb < 4:
                    return k_loc[:, hk, kb * 128:(kb + 1) * 128]
                rb = kb - 4
                return hk_tiles[hk][0][:, rb // 4, (rb % 4) * 128:(rb % 4 + 1) * 128]

            def vchunk(hk, kb):
                if kb < 4:
                    return vp_loc[:, hk * 4 + kb, :]
                return hk_tiles[hk][1][:, kb - 4, :]

            ops_of = {}

            def emit_local_done(hi):
                # spill the local partial (numerator + ones-col denominator)
                # to SBUF, freeing the PSUM banks for the next head
                ops = ops_of.pop(hi)
                for i in range(2):
                    nc.vector.tensor_copy(
                        lpart_sb[:, hi, 2 * i:2 * i + 2, :], ops[i][:])

            def emit_norm(hi):
                _, h = heads[hi]
                ops = ops_of.pop(hi)
                o_part = opool.tile([128, 4, HD], BF16, name="o_part")
                for i in range(2):
                    mrg = orpool.tile([128, 2, HD + 1], F32, name=f"mrg{i}")
                    nc.vector.tensor_tensor(
                        mrg[:], ops[i][:],
                        lpart_sb[:, hi, 2 * i:2 * i + 2, :], ADD)
                    rin = npool.tile([128, 2, 1], F32, name=f"rin{i}")
                    nc.vector.reciprocal(rin[:], mrg[:, :, HD:HD + 1])
                    for j in range(2):
                        qc = i * 2 + j
                        nc.vector.tensor_scalar_mul(
                            o_part[:, qc, :],
                            mrg[:, j, 0:HD], rin[:, j, :])
                nc.sync.dma_start_transpose(oT_sb[h][:], o_part[:])

            def emit_one_pv(item, opener, closer):
                pvt, j, hi, kb = item
                hk, _ = heads[hi]
                for qc in range(4):
                    nc.tensor.matmul(
                        ops_of[hi][qc // 2][:, qc % 2, :],
                        pvt[:, j, qc * 128:(qc + 1) * 128],
                        vchunk(hk, kb),
                        start=(kb == opener and qc % 2 == 0),
                        stop=(kb == closer))
                if kb == closer:
                    if closer == 3:
                        emit_local_done(hi)
                    else:
                        emit_norm(hi)

            from collections import deque
            PV_LAG = 6
            st_ctr = [0]

            def run_stream(units, opener, closer, first_pass):
                pvq = deque()
                chunks = [tuple(units[i:i + 3])
                          for i in range(0, len(units), 3)]
                for chunk in chunks:
                    for hi, kb in chunk:
                        if kb == opener:
                            ops_of[hi] = [
                                ops_pool.tile([128, 2, HD + 1], F32,
                                              name=f"ops{i}")
                                for i in range(2)]
                            hk = heads[hi][0]
                            if not first_pass and hi % G == 0:
                                # wo chunks emitted in the REMOTE pass: they
                                # queue behind the gather reads but ahead of
                                # only a few oT transposes at a time, so the
                                # o_part recycle never dams behind 8MB of wo
                                nc.sync.dma_start(
                                    wo_sb[:, hk * 4:(hk + 1) * 4, :],
                                    wo_d[hk * 4:(hk + 1) * 4]
                                    .rearrange("h p c -> p h c"))
                    st = stt[st_ctr[0] % 2]
                    st_ctr[0] += 1
                    for j, (hi, kb) in enumerate(chunk):
                        hk, h = heads[hi]
                        nc.tensor.matmul(st[:, j, :], kchunk(hk, kb),
                                         q_sb[h][:], start=True, stop=True)
                        if len(pvq) > PV_LAG:
                            emit_one_pv(pvq.popleft(), opener, closer)
                    pt = ptpool.tile([128, 3, NQ], BF16, name="pt_t")
                    nj = len(chunk)
                    nc.scalar.activation(pt[:, 0:nj, :], st[:, 0:nj, :],
                                         EXP, scale=SCALE)
                    for j, (hi, kb) in enumerate(chunk):
                        pvq.append((pt, j, hi, kb))
                while pvq:
                    emit_one_pv(pvq.popleft(), opener, closer)

            run_stream([(hi, kb) for hi in range(len(heads))
                        for kb in range(4)], 0, 3, True)
            run_stream([(hi, kb) for hi in range(len(heads))
                        for kb in range(4, KB)], 4, KB - 1, False)

        # ---------------- Output projection (weights already resident) --------
        with ExitStack() as ph:
            outsb = ph.enter_context(tc.tile_pool(name="outsb", bufs=4))
            outps = ph.enter_context(tc.tile_pool(name="outps", bufs=3, space="PSUM"))
            for dcol in range(4):
                for qc in range(4):
                    outp = outps.tile([128, 512], F32, name="outp")
                    for h in range(H):
                        nc.tensor.matmul(
                            outp[:], oT_sb[h][:, qc, :],
                            wo_sb[:, h, dcol * 512:(dcol + 1) * 512],
                            start=(h == 0), stop=(h == H - 1))
                    osb = outsb.tile([128, 512], BF16, name="osb")
                    nc.vector.tensor_copy(osb[:], outp[:])
                    nc.sync.dma_start(
                        out_d[qc * 128:(qc + 1) * 128,
                              dcol * 512:(dcol + 1) * 512], osb[:])

    nc.compile()
    return nc


def _prep_inputs(x, cos, sin, Wq, Wkv, Wo):
    bf = ml_dtypes.bfloat16
    wq_prep = np.ascontiguousarray(
        Wq.reshape(DC, 128, H, HD).transpose(2, 1, 0, 3)).astype(bf)
    wk_prep = np.ascontiguousarray(
        Wkv[:, :HKV * HD].reshape(DC, 128, HKV, HD).transpose(2, 1, 0, 3)).astype(bf)
    wv_prep = np.ascontiguousarray(
        Wkv[:, HKV * HD:].reshape(DC, 128, HKV * HD)).astype(bf)
    wo_prep = np.ascontiguousarray(Wo.reshape(H, HD, D)).astype(bf)
    c64 = cos[0, :, 0, :].T.astype(np.float32)   # [64, N]
    s64 = sin[0, :, 0, :].T.astype(np.float32)
    cosT = np.ascontiguousarray(np.concatenate([c64, c64], axis=0))   # [128, N]
    sinT = np.ascontiguousarray(np.concatenate([s64, -s64], axis=0))

    in_maps = []
    for c in range(NCORES):
        b, j = divmod(c, 4)
        rows = slice(j * NQ, (j + 1) * NQ)
        xT = np.ascontiguousarray(x[b].T[:, rows]).astype(bf)
        in_maps.append({
            "xT": xT,
            "cosT": np.ascontiguousarray(cosT[:, rows]),
            "sinT": np.ascontiguousarray(sinT[:, rows]),
            "wq": wq_prep, "wk": wk_prep, "wv": wv_prep, "wo": wo_prep,
        })
    return in_maps


def kernel(x, cos, sin, attn_mask, Wq, Wkv, Wo, bo):
    x = np.asarray(x, np.float32)
    cos = np.asarray(cos, np.float32)
    sin = np.asarray(sin, np.float32)
    Wq = np.asarray(Wq, np.float32)
    Wkv = np.asarray(Wkv, np.float32)
    Wo = np.asarray(Wo, np.float32)
    bo = np.asarray(bo, np.float32)

    if "nc" not in _cache:
        _cache["nc"] = _build()
    nc = _cache["nc"]

    in_maps = _prep_inputs(x, cos, sin, Wq, Wkv, Wo)
    res = run_bass_kernel_spmd(nc, in_maps, list(range(NCORES)))
    out = np.empty((B, N, D), np.float32)
    for c in range(NCORES):
        b, j = divmod(c, 4)
        out[b, j * NQ:(j + 1) * NQ, :] = np.asarray(
            res.results[c]["out"], dtype=np.float32)
    out += bo[None, None, :]
    return out


# revision 39
# speedup vs baseline: 1.0287x; 1.0287x over previous
"""GQA attention block (B=2, N=2048, D=2048, H=16, HKV=4, HD=128) on 8 TRN2 cores.

Sharding: core c -> batch b = c // 4, query-row quarter j = c % 4 (512 rows).

The HAM-sustained PE clock is 13/16 * 2.4GHz = 1.946 GHz (k=13/n=16 in the
ham trace records), so the bf16 PE-stream floor is ~304us/core; the schedule
packs every phase against it:
  - DMA issue order == consumption order, fine-chunked so the first K
    matmul needs only ~2.5MB; wq groups 0-1 ride the early stream, groups
    2-3 go after the bounce writes (outside the AllGather's position-based
    readiness gate, streamed inside its trigger delay).
  - K projection per kv head (16 contraction chunks into 4 PSUM banks),
    RoPE(K) + scalar-ring bounce per head; V projection n4-outer with the
    first accumulator on the spare bank so nothing waits on the (pool-
    coarse) WAR against the K RoPE PSUM reads.
  - One fused AllGather of K+V within each 4-core batch group.  Its data
    phase completes only after the SLOWEST peer triggers -- peer launch
    skew is 30-100us -- so everything that depends on it is scheduled as
    late as possible: attention runs in TWO passes, first every head's 4
    LOCAL key blocks (~39us, no collective dependency, partials spilled to
    SBUF), then the 12 remote blocks with a partial-merge at normalize.
  - All gather reads are emitted right after the collective (they fire the
    moment it lands); wo streams during the remote pass behind them.
  - attention in transposed-score form: S^T = K.Q^T, exp on ScalarE (~128us
    busy: 131072 elems/partition at 1.2GHz), keys grouped in triples per
    exp op; PV lag 6.  Denominator via ones-column appended to V.
  - PSUM bank layout is controlled by pool-entry order (kvps 0-3, vsp 4,
    qps 5-7; attention stA 0-2, ops 3-4, stB 5-7) so pool-coarse WAR
    syncs against the last Q heads' RoPE reads land on the second score
    chunk / never on the critical edge.
  - q_sb and oT are per-head tiles: tile-granular RAW lets the first score
    start after RoPE(h0) (not all 16) and the output projection pipeline
    behind the per-head XBAR transposes (one batched [128,4,128] transpose
    per head).
  - normalization multiplies by DVE-reciprocal of the merged denominator;
    output projection accumulates all 16 heads; bf16 output DMA (host
    widens to f32 and adds bo).
All matmuls bf16 with f32 PSUM accumulation; softmax statistics in f32.
A burst of zero-weight matmuls at kernel start warms the PE HAM clock gate
(cold PE runs at 1.2 GHz) while the first input DMAs are still in flight.
"""

import numpy as np
import ml_dtypes

import concourse.bass as bass
import concourse.mybir as mybir
import concourse.tile as tile
from concourse import bacc
from concourse.bass_utils import run_bass_kernel_spmd

B, N, D = 2, 2048, 2048
H, HKV, HD = 16, 4, 128
G = H // HKV
NQ = N // 4          # query rows per core
DC = D // 128        # contraction chunks for projections
KB = N // 128        # key blocks per batch
NCORES = 8
SCALE = float(HD) ** -0.5

BF16 = mybir.dt.bfloat16
F32 = mybir.dt.float32
MUL = mybir.AluOpType.mult
ADD = mybir.AluOpType.add
EXP = mybir.ActivationFunctionType.Exp

_cache = {}


def _rope(nc, pool, out, in_psum, cos2_sb, sin2_sb):
    """Rotate-half RoPE with head-dim on partitions.

    cos2_sb = [cos; cos], sin2_sb = [sin; -sin] (128 rows, host-prepared), so
    out = t*cos2 + rot(t)*sin2 where rot swaps the partition halves.
    ScalarE (idle during projections) does the PSUM reads; the three DVE
    multiplies/adds then run all-SBUF at the 2x f32 rate.
    """
    rot = pool.tile([128, NQ], F32, name="rope_rot")
    nc.scalar.copy(rot[0:64, :], in_psum[64:128, :])
    nc.scalar.copy(rot[64:128, :], in_psum[0:64, :])
    m1 = pool.tile([128, NQ], F32, name="rope_m1")
    m2 = pool.tile([128, NQ], F32, name="rope_m2")
    nc.vector.tensor_tensor(m1[:], in_psum[:], cos2_sb[:], MUL)
    nc.vector.tensor_tensor(m2[:], rot[:], sin2_sb[:], MUL)
    nc.vector.tensor_tensor(out[:], m1[:], m2[:], ADD)


def _build():
    from contextlib import ExitStack

    nc = bacc.Bacc("TRN2", target_bir_lowering=False, debug=False,
                   num_devices=NCORES)

    xT_d = nc.dram_tensor("xT", [D, NQ], BF16, kind="ExternalInput").ap()
    cosT_d = nc.dram_tensor("cosT", [HD, NQ], F32, kind="ExternalInput").ap()
    sinT_d = nc.dram_tensor("sinT", [HD, NQ], F32, kind="ExternalInput").ap()
    wq_d = nc.dram_tensor("wq", [H, 128, DC, 128], BF16, kind="ExternalInput").ap()
    wk_d = nc.dram_tensor("wk", [HKV, 128, DC, 128], BF16, kind="ExternalInput").ap()
    wv_d = nc.dram_tensor("wv", [DC, 128, HKV * HD], BF16, kind="ExternalInput").ap()
    wo_d = nc.dram_tensor("wo", [H, 128, D], BF16, kind="ExternalInput").ap()
    out_d = nc.dram_tensor("out", [NQ, D], BF16, kind="ExternalOutput").ap()

    with tile.TileContext(nc) as tc, ExitStack() as top:
        resident = top.enter_context(tc.tile_pool(name="resident", bufs=1))
        dram = top.enter_context(tc.tile_pool(name="dram", bufs=1, space="DRAM"))
        # remote K/V tiles outlive the projection scope -> enter before it;
        # bufs=4 holds every kv head at once so no gather read is ever
        # WAR-blocked at the head of the DMA ring (damming traffic behind it)
        ktpool = top.enter_context(tc.tile_pool(name="ktpool", bufs=4))
        vppool = top.enter_context(tc.tile_pool(name="vppool", bufs=4))

        warmw = resident.tile([128, 128], BF16)
        nc.gpsimd.memset(warmw[:], 0.0)

        # HAM warm-up: the PE clock gate defaults to 1.2 GHz and needs ~3.4us
        # of sustained matmul activity to lift.  These zero-weight matmuls
        # depend on no DMA, so they run while the first input chunks are
        # still in flight and the real matmul stream starts warm.
        with tc.tile_pool(name="warm", bufs=1, space="PSUM") as wp:
            wps = wp.tile([128, 128], F32)
            for _ in range(64):
                nc.tensor.matmul(wps[:], warmw[:], warmw[:],
                                 start=True, stop=True)

        q_sb = [resident.tile([128, NQ], BF16, name=f"q_sb{_h}")
                for _h in range(H)]
        k_loc = resident.tile([128, HKV, NQ], BF16)       # roped local K, d-major
        vp_loc = resident.tile([128, HKV * 4, HD + 1], BF16)  # (hk, local kb)
        nc.gpsimd.memset(vp_loc[:, :, HD:HD + 1], 1.0)
        oT_sb = [resident.tile([128, 4, 128], BF16, name=f"oT{_h}")
                 for _h in range(H)]

        # kv bounce: rows 0..511 = roped K (4 heads x 128 d), cols = local n;
        # rows 512..1023 = V (local n rows), cols = 4 heads x 128 channels
        kv_bounce = dram.tile([2 * NQ, NQ], BF16)
        ag_out = dram.tile([2 * NQ * 4, NQ], BF16)

        # -- projection scope: tensors freed after the Q phase ------------------
        proj_scope = ExitStack()
        proj = proj_scope.enter_context(tc.tile_pool(name="proj", bufs=1))
        tmp_pool = proj_scope.enter_context(tc.tile_pool(name="ropetmp", bufs=3))
        cos_sb = proj.tile([HD, NQ], F32)
        sin_sb = proj.tile([HD, NQ], F32)
        xts = proj.tile([128, DC, NQ], BF16)
        xT_r = xT_d.rearrange("(dc p) n -> p dc n", p=128)
        wq_sb = proj.tile([128, H, DC * 128], BF16)

        # ---------------- KV projection + RoPE(K) + bounce-out ----------------
        with ExitStack() as ph:
            wkpool = ph.enter_context(tc.tile_pool(name="wkpool", bufs=1))
            wvpool = ph.enter_context(tc.tile_pool(name="wvpool", bufs=1))
            kvsb = ph.enter_context(tc.tile_pool(name="kvsb", bufs=3))
            # PSUM entry order fixes bank addresses: kvps 0-3, vsp 4,
            # qps 5-7.  The attention pools then overlap as stA 0-2 (K
            # banks, drained early), ops 3-4 (K/V banks), stB 5-7 (the qps
            # banks, whose pool-coarse WAR on the last Q RoPE reads lands
            # on the SECOND score chunk, ~1.6us late by construction).
            kvps_pool = proj_scope.enter_context(
                tc.tile_pool(name="kvps", bufs=4, space="PSUM"))
            vsp_pool = proj_scope.enter_context(
                tc.tile_pool(name="vsp", bufs=1, space="PSUM"))
            qps_pool = proj_scope.enter_context(
                tc.tile_pool(name="qps", bufs=1, space="PSUM"))

            wk_sb = wkpool.tile([128, HKV, DC, 128], BF16)
            wv_sb = wvpool.tile([128, DC, HKV * HD], BF16)
            wq_r = wq_d.rearrange("h p dc c -> p h (dc c)")
            wk_r = wk_d.rearrange("hk p dc c -> p hk (dc c)")

            # DMA issue order == compute consumption order (the ring drains
            # in order).  Fine chunks up front so the first K matmul needs
            # only ~2.5MB; wq groups 0-1 ride the early stream (Q proj
            # starts the moment the KV phase drains); wq groups 2-3 are
            # emitted after the bounce writes, excluded from the AllGather's
            # position-based readiness gate, and stream inside its trigger
            # delay, clear of the AG data phase.
            nc.sync.dma_start(wk_sb[:, 0:1, :, :], wk_r[:, 0:1, :])
            nc.sync.dma_start(xts[:, 0:4, :], xT_r[:, 0:4, :])
            nc.sync.dma_start(xts[:, 4:8, :], xT_r[:, 4:8, :])
            nc.sync.dma_start(xts[:, 8:12, :], xT_r[:, 8:12, :])
            nc.sync.dma_start(xts[:, 12:16, :], xT_r[:, 12:16, :])
            nc.sync.dma_start(wk_sb[:, 1:4, :, :], wk_r[:, 1:4, :])
            nc.sync.dma_start(wv_sb[:], wv_d.rearrange("d p c -> p d c"))
            nc.sync.dma_start(cos_sb[:], cosT_d)
            nc.sync.dma_start(sin_sb[:], sinT_d)
            nc.sync.dma_start(wq_sb[:, 0:4, :], wq_r[:, 0:4, :])
            nc.sync.dma_start(wq_sb[:, 4:8, :], wq_r[:, 4:8, :])

            for hk in range(HKV):
                kps = kvps_pool.tile([128, NQ], F32, name="kvps_t")
                for dc in range(DC):
                    nc.tensor.matmul(kps[:], wk_sb[:, hk, dc, :], xts[:, dc, :],
                                     start=(dc == 0), stop=(dc == DC - 1))
                _rope(nc, tmp_pool, k_loc[:, hk, :], kps, cos_sb, sin_sb)
                # scalar-ring DMA: jumps ahead of bulk traffic on the sync
                # ring so the AllGather can start as soon as K/V are ready
                nc.scalar.dma_start(kv_bounce[hk * 128:(hk + 1) * 128, :],
                                    k_loc[:, hk, :])

            # n4-outer with the first tile on the spare PSUM bank: the first
            # V chain starts with no WAR at all, and by the time chain n4=1
            # wants a kvps bank the K RoPE reads (pool-coarse WAR) have
            # drained behind chain n4=0's matmuls.
            vps_tiles = [vsp_pool.tile([128, HKV * HD], F32, name="vsp_t")]
            vps_tiles += [kvps_pool.tile([128, HKV * HD], F32, name="kvps_t")
                          for i in range(3)]
            for n4 in range(4):
                for dc in range(DC):
                    nc.tensor.matmul(
                        vps_tiles[n4][:],
                        xts[:, dc, n4 * 128:(n4 + 1) * 128],
                        wv_sb[:, dc, :],
                        start=(dc == 0), stop=(dc == DC - 1))
            for n4 in range(4):
                v_sb = kvsb.tile([128, HKV * HD], BF16, name="v_sb")
                nc.vector.tensor_copy(v_sb[:], vps_tiles[n4][:])
                nc.scalar.dma_start(
                    kv_bounce[NQ + n4 * 128:NQ + (n4 + 1) * 128, :], v_sb[:])
                for hk in range(HKV):
                    nc.vector.tensor_copy(
                        vp_loc[:, hk * 4 + n4, 0:HD],
                        vps_tiles[n4][:, hk * HD:(hk + 1) * HD])

            # wq groups 2-3 emitted AFTER the bounce DMAs: excluded from the
            # collective's readiness gate, streamed during its trigger delay
            for q4 in range(2, 4):
                nc.sync.dma_start(wq_sb[:, q4 * 4:(q4 + 1) * 4, :],
                                  wq_r[:, q4 * 4:(q4 + 1) * 4, :])

        # ---------------- AllGather K,V within the batch group ----------------
        # Single fused collective (split collectives serialize on the CC core).
        nc.gpsimd.collective_compute(
            "AllGather", mybir.AluOpType.bypass,
            replica_groups=[[0, 1, 2, 3], [4, 5, 6, 7]],
            ins=[kv_bounce.opt()],
            outs=[ag_out.opt()],
        )

        # All four kv-heads' gather reads are emitted NOW so the DMAs fire
        # the moment the collective lands (during Q projection) instead of
        # at attention start.
        pid = nc.sync.partition_id()
        slots = [(pid + i) % 4 for i in (1, 2, 3)]
        hk_tiles = {}

        def load_kv_tiles(hk):
            ktr = ktpool.tile([128, 3, NQ], BF16, name="ktr")
            for i, slot in enumerate(slots):
                nc.sync.dma_start(
                    ktr[:, i, :],
                    ag_out[bass.ds(slot * 2 * NQ + hk * 128, 128), :])
            vpr = vppool.tile([128, 12, HD + 1], BF16, name="vpr")
            nc.gpsimd.memset(vpr[:, :, HD:HD + 1], 1.0)
            for i, slot in enumerate(slots):
                src = ag_out[bass.ds(slot * 2 * NQ + NQ, NQ),
                             hk * HD:(hk + 1) * HD]
                nc.sync.dma_start(
                    vpr[:, i * 4:(i + 1) * 4, 0:HD],
                    src.rearrange("(kbl p) c -> p kbl c", p=128))
            hk_tiles[hk] = (ktr, vpr)

        for _hk in range(HKV):
            load_kv_tiles(_hk)

        # ---------------- Q projection + RoPE ---------------------------------
        # Manual qps rotation (15-h)%3: bank 2 (first st bank the attention
        # scores need) is last touched by h13's RoPE, banks 0-1 (the ops
        # banks, needed a few us later) by h15/h14 -> no attention-start WAR.
        qts = [qps_pool.tile([128, NQ], F32, name=f"qps_t{i}")
               for i in range(3)]
        for h in range(H):
            qps = qts[(15 - h) % 3]
            for dc in range(DC):
                nc.tensor.matmul(qps[:],
                                 wq_sb[:, h, dc * 128:(dc + 1) * 128],
                                 xts[:, dc, :],
                                 start=(dc == 0), stop=(dc == DC - 1))
            _rope(nc, tmp_pool, q_sb[h][:], qps, cos_sb, sin_sb)
        proj_scope.close()

        # Wo lands in the space freed by the projection tensors; its DMAs are
        # emitted inside the attention loop (per kv head) so they queue on
        # the ring behind the AG-gated gather reads and never contend with
        # the collective.
        post = top.enter_context(tc.tile_pool(name="post", bufs=1))
        wo_sb = post.tile([128, H, D], BF16)

        # ---------------- Attention (flat pipelined stream) -------------------
        # One uniform stream of (head, key-block) pairs in triples: each exp
        # op spans head boundaries, so ScalarE (the attention-phase floor)
        # runs continuously and head flushes never stall the PE.
        # PV accumulates 16 blocks per head into two packed 2-slot PSUM tiles
        # (start=True only opens the first slot's chain: it clears has_written
        # for the whole bank, so the second chain opens with start=False on
        # freshly-cleared bits).  Each head's PSUM banks are released by a
        # short raw DVE copy; reciprocal + normalize + the [128,128] XBAR
        # transposes run afterwards, off the inter-head chain.
        # The stream runs in TWO passes to absorb peer-core launch skew (the
        # AllGather's data phase ends only after the slowest peer in the
        # replica group triggers, which can be 30-45us after this core):
        # pass 1 covers every head's LOCAL key blocks (kb 0-3, no collective
        # dependency, ~37us of PE work), spilling each head's partial
        # numerator/denominator to SBUF; pass 2 streams the remote blocks
        # and merges the partials during normalization.
        with ExitStack() as ph:
            ptpool = ph.enter_context(tc.tile_pool(name="ptpool", bufs=5))
            npool = ph.enter_context(tc.tile_pool(name="npool", bufs=4))
            orpool = ph.enter_context(tc.tile_pool(name="orpool", bufs=3))
            opool = ph.enter_context(tc.tile_pool(name="opool", bufs=6))
            lppool = ph.enter_context(tc.tile_pool(name="lppool", bufs=1))
            # all 16 heads' local partials live until pass 2 -> one static tile
            lpart_sb = lppool.tile([128, H, 4, HD + 1], BF16)
            stA_pool = ph.enter_context(tc.tile_pool(name="stA", bufs=1, space="PSUM"))
            ops_pool = ph.enter_context(tc.tile_pool(name="opsp", bufs=1, space="PSUM"))
            stB_pool = ph.enter_context(tc.tile_pool(name="stB", bufs=1, space="PSUM"))
            stt = [stA_pool.tile([128, 3, NQ], F32, name="stA0"),
                   stB_pool.tile([128, 3, NQ], F32, name="stB0")]

            heads = [(hk, g * HKV + hk) for hk in range(HKV) for g in range(G)]

            def kchunk(hk, kb):
                if kb < 4:
                    return k_loc[:, hk, kb * 128:(kb + 1) * 128]
                rb = kb - 4
                return hk_tiles[hk][0][:, rb // 4, (rb % 4) * 128:(rb % 4 + 1) * 128]

            def vchunk(hk, kb):
                if kb < 4:
                    return vp_loc[:, hk * 4 + kb, :]
                return hk_tiles[hk][1][:, kb - 4, :]

            ops_of = {}

            def emit_local_done(hi):
                # spill the local partial (numerator + ones-col denominator)
                # to SBUF, freeing the PSUM banks for the next head
                ops = ops_of.pop(hi)
                for i in range(2):
                    nc.vector.tensor_copy(
                        lpart_sb[:, hi, 2 * i:2 * i + 2, :], ops[i][:])

            def emit_norm(hi):
                _, h = heads[hi]
                ops = ops_of.pop(hi)
                o_part = opool.tile([128, 4, HD], BF16, name="o_part")
                for i in range(2):
                    mrg = orpool.tile([128, 2, HD + 1], F32, name=f"mrg{i}")
                    nc.vector.tensor_tensor(
                        mrg[:], ops[i][:],
                        lpart_sb[:, hi, 2 * i:2 * i + 2, :], ADD)
                    rin = npool.tile([128, 2, 1], F32, name=f"rin{i}")
                    nc.vector.reciprocal(rin[:], mrg[:, :, HD:HD + 1])
                    for j in range(2):
                        qc = i * 2 + j
                        nc.vector.tensor_scalar_mul(
                            o_part[:, qc, :],
                            mrg[:, j, 0:HD], rin[:, j, :])
                nc.sync.dma_start_transpose(oT_sb[h][:], o_part[:])

            def emit_one_pv(item, opener, closer):
                pvt, j, hi, kb = item
                hk, _ = heads[hi]
                for qc in range(4):
                    nc.tensor.matmul(
                        ops_of[hi][qc // 2][:, qc % 2, :],
                        pvt[:, j, qc * 128:(qc + 1) * 128],
                        vchunk(hk, kb),
                        start=(kb == opener and qc % 2 == 0),
                        stop=(kb == closer))
                if kb == closer:
                    if closer == 3:
                        emit_local_done(hi)
                    else:
                        emit_norm(hi)

            from collections import deque
            PV_LAG = 6
            st_ctr = [0]

            def run_stream(units, opener, closer, first_pass):
                pvq = deque()
                chunks = [tuple(units[i:i + 3])
                          for i in range(0, len(units), 3)]
                for chunk in chunks:
                    for hi, kb in chunk:
                        if kb == opener:
                            ops_of[hi] = [
                                ops_pool.tile([128, 2, HD + 1], F32,
                                              name=f"ops{i}")
                                for i in range(2)]
                            hk = heads[hi][0]
                            if not first_pass and hi % G == 0:
                                # wo chunks emitted in the REMOTE pass: they
                                # queue behind the gather reads but ahead of
                                # only a few oT transposes at a time, so the
                                # o_part recycle never dams behind 8MB of wo
                                nc.sync.dma_start(
                                    wo_sb[:, hk * 4:(hk + 1) * 4, :],
                                    wo_d[hk * 4:(hk + 1) * 4]
                                    .rearrange("h p c -> p h c"))
                    st = stt[st_ctr[0] % 2]
                    st_ctr[0] += 1
                    for j, (hi, kb) in enumerate(chunk):
                        hk, h = heads[hi]
                        nc.tensor.matmul(st[:, j, :], kchunk(hk, kb),
                                         q_sb[h][:], start=True, stop=True)
                        if len(pvq) > PV_LAG:
                            emit_one_pv(pvq.popleft(), opener, closer)
                    pt = ptpool.tile([128, 3, NQ], BF16, name="pt_t")
                    nj = len(chunk)
                    nc.scalar.activation(pt[:, 0:nj, :], st[:, 0:nj, :],
                                         EXP, scale=SCALE)
                    for j, (hi, kb) in enumerate(chunk):
                        pvq.append((pt, j, hi, kb))
                while pvq:
                    emit_one_pv(pvq.popleft(), opener, closer)

            run_stream([(hi, kb) for hi in range(len(heads))
                        for kb in range(4)], 0, 3, True)
            run_stream([(hi, kb) for hi in range(len(heads))
                        for kb in range(4, KB)], 4, KB - 1, False)

        # ---------------- Output projection (weights already resident) --------
        with ExitStack() as ph:
            outsb = ph.enter_context(tc.tile_pool(name="outsb", bufs=4))
            outps = ph.enter_context(tc.tile_pool(name="outps", bufs=3, space="PSUM"))
            for dcol in range(4):
                for qc in range(4):
                    outp = outps.tile([128, 512], F32, name="outp")
                    for h in range(H):
                        nc.tensor.matmul(
                            outp[:], oT_sb[h][:, qc, :],
                            wo_sb[:, h, dcol * 512:(dcol + 1) * 512],
                            start=(h == 0), stop=(h == H - 1))
                    osb = outsb.tile([128, 512], BF16, name="osb")
                    nc.vector.tensor_copy(osb[:], outp[:])
                    nc.sync.dma_start(
                        out_d[qc * 128:(qc + 1) * 128,
                              dcol * 512:(dcol + 1) * 512], osb[:])

    nc.compile()
    return nc


def _prep_inputs(x, cos, sin, Wq, Wkv, Wo):
    bf = ml_dtypes.bfloat16
    wq_prep = np.ascontiguousarray(
        Wq.reshape(DC, 128, H, HD).transpose(2, 1, 0, 3)).astype(bf)
    wk_prep = np.ascontiguousarray(
        Wkv[:, :HKV * HD].reshape(DC, 128, HKV, HD).transpose(2, 1, 0, 3)).astype(bf)
    wv_prep = np.ascontiguousarray(
        Wkv[:, HKV * HD:].reshape(DC, 128, HKV * HD)).astype(bf)
    wo_prep = np.ascontiguousarray(Wo.reshape(H, HD, D)).astype(bf)
    c64 = cos[0, :, 0, :].T.astype(np.float32)   # [64, N]
    s64 = sin[0, :, 0, :].T.astype(np.float32)
    cosT = np.ascontiguousarray(np.concatenate([c64, c64], axis=0))   # [128, N]
    sinT = np.ascontiguousarray(np.concatenate([s64, -s64], axis=0))

    in_maps = []
    for c in range(NCORES):
        b, j = divmod(c, 4)
        rows = slice(j * NQ, (j + 1) * NQ)
        xT = np.ascontiguousarray(x[b].T[:, rows]).astype(bf)
        in_maps.append({
            "xT": xT,
            "cosT": np.ascontiguousarray(cosT[:, rows]),
            "sinT": np.ascontiguousarray(sinT[:, rows]),
            "wq": wq_prep, "wk": wk_prep, "wv": wv_prep, "wo": wo_prep,
        })
    return in_maps


def kernel(x, cos, sin, attn_mask, Wq, Wkv, Wo, bo):
    x = np.asarray(x, np.float32)
    cos = np.asarray(cos, np.float32)
    sin = np.asarray(sin, np.float32)
    Wq = np.asarray(Wq, np.float32)
    Wkv = np.asarray(Wkv, np.float32)
    Wo = np.asarray(Wo, np.float32)
    bo = np.asarray(bo, np.float32)

    if "nc" not in _cache:
        _cache["nc"] = _build()
    nc = _cache["nc"]

    in_maps = _prep_inputs(x, cos, sin, Wq, Wkv, Wo)
    res = run_bass_kernel_spmd(nc, in_maps, list(range(NCORES)))
    out = np.empty((B, N, D), np.float32)
    for c in range(NCORES):
        b, j = divmod(c, 4)
        out[b, j * NQ:(j + 1) * NQ, :] = np.asarray(
            res.results[c]["out"], dtype=np.float32)
    out += bo[None, None, :]
    return out


# revision 40
# speedup vs baseline: 1.0511x; 1.0218x over previous
"""GQA attention block (B=2, N=2048, D=2048, H=16, HKV=4, HD=128) on 8 TRN2 cores.

Sharding: core c -> batch b = c // 4, query-row quarter j = c % 4 (512 rows).

The HAM-sustained PE clock is 13/16 * 2.4GHz = 1.946 GHz (k=13/n=16 in the
ham trace records), so the bf16 PE-stream floor is ~304us/core; the schedule
packs every phase against it:
  - DMA issue order == consumption order, fine-chunked so the first K
    matmul needs only ~2.5MB; wq groups 0-1 ride the early stream, groups
    2-3 go after the bounce writes (outside the AllGather's position-based
    readiness gate, streamed inside its trigger delay).
  - K projection per kv head (16 contraction chunks into 4 PSUM banks),
    RoPE(K) + scalar-ring bounce per head; V projection n4-outer with the
    first accumulator on the spare bank so nothing waits on the (pool-
    coarse) WAR against the K RoPE PSUM reads.
  - One fused AllGather of K+V within each 4-core batch group.  Its data
    phase completes only after the SLOWEST peer triggers -- peer launch
    skew is 30-100us -- so everything that depends on it is scheduled as
    late as possible: attention runs in TWO passes, first every head's 4
    LOCAL key blocks (~39us, no collective dependency, partials spilled to
    SBUF), then the 12 remote blocks with a partial-merge at normalize.
  - All gather reads are emitted right after the collective (they fire the
    moment it lands); wo streams during the remote pass behind them.
  - attention in transposed-score form: S^T = K.Q^T, exp on ScalarE (~128us
    busy: 131072 elems/partition at 1.2GHz), keys grouped in triples per
    exp op; PV lag 6.  Denominator via ones-column appended to V.
  - PSUM bank layout is controlled by pool-entry order (kvps 0-3, vsp 4,
    qps 5-7; attention stA 0-2, ops 3-4, stB 5-7) so pool-coarse WAR
    syncs against the last Q heads' RoPE reads land on the second score
    chunk / never on the critical edge.
  - q_sb and oT are per-head tiles: tile-granular RAW lets the first score
    start after RoPE(h0) (not all 16) and the output projection pipeline
    behind the per-head XBAR transposes (one batched [128,4,128] transpose
    per head).
  - normalization multiplies by DVE-reciprocal of the merged denominator;
    output projection accumulates all 16 heads; bf16 output DMA (host
    widens to f32 and adds bo).
All matmuls bf16 with f32 PSUM accumulation; softmax statistics in f32.
A burst of zero-weight matmuls at kernel start warms the PE HAM clock gate
(cold PE runs at 1.2 GHz) while the first input DMAs are still in flight.
"""

import numpy as np
import ml_dtypes

import concourse.bass as bass
import concourse.mybir as mybir
import concourse.tile as tile
from concourse import bacc
from concourse.bass_utils import run_bass_kernel_spmd

B, N, D = 2, 2048, 2048
H, HKV, HD = 16, 4, 128
G = H // HKV
NQ = N // 4          # query rows per core
DC = D // 128        # contraction chunks for projections
KB = N // 128        # key blocks per batch
NCORES = 8
SCALE = float(HD) ** -0.5

BF16 = mybir.dt.bfloat16
F32 = mybir.dt.float32
MUL = mybir.AluOpType.mult
ADD = mybir.AluOpType.add
EXP = mybir.ActivationFunctionType.Exp

_cache = {}


def _rope(nc, pool, out, in_psum, cos2_sb, sin2_sb):
    """Rotate-half RoPE with head-dim on partitions.

    cos2_sb = [cos; cos], sin2_sb = [sin; -sin] (128 rows, host-prepared), so
    out = t*cos2 + rot(t)*sin2 where rot swaps the partition halves.
    ScalarE (idle during projections) does the PSUM reads; the three DVE
    multiplies/adds then run all-SBUF at the 2x f32 rate.
    """
    rot = pool.tile([128, NQ], F32, name="rope_rot")
    nc.scalar.copy(rot[0:64, :], in_psum[64:128, :])
    nc.scalar.copy(rot[64:128, :], in_psum[0:64, :])
    m1 = pool.tile([128, NQ], F32, name="rope_m1")
    m2 = pool.tile([128, NQ], F32, name="rope_m2")
    nc.vector.tensor_tensor(m1[:], in_psum[:], cos2_sb[:], MUL)
    nc.vector.tensor_tensor(m2[:], rot[:], sin2_sb[:], MUL)
    nc.vector.tensor_tensor(out[:], m1[:], m2[:], ADD)


def _build():
    from contextlib import ExitStack

    nc = bacc.Bacc("TRN2", target_bir_lowering=False, debug=False,
                   num_devices=NCORES)

    xT_d = nc.dram_tensor("xT", [D, NQ], BF16, kind="ExternalInput").ap()
    cosT_d = nc.dram_tensor("cosT", [HD, NQ], F32, kind="ExternalInput").ap()
    sinT_d = nc.dram_tensor("sinT", [HD, NQ], F32, kind="ExternalInput").ap()
    wq_d = nc.dram_tensor("wq", [H, 128, DC, 128], BF16, kind="ExternalInput").ap()
    wk_d = nc.dram_tensor("wk", [HKV, 128, DC, 128], BF16, kind="ExternalInput").ap()
    wv_d = nc.dram_tensor("wv", [DC, 128, HKV * HD], BF16, kind="ExternalInput").ap()
    wo_d = nc.dram_tensor("wo", [H, 128, D], BF16, kind="ExternalInput").ap()
    out_d = nc.dram_tensor("out", [NQ, D], BF16, kind="ExternalOutput").ap()

    with tile.TileContext(nc) as tc, ExitStack() as top:
        resident = top.enter_context(tc.tile_pool(name="resident", bufs=1))
        dram = top.enter_context(tc.tile_pool(name="dram", bufs=1, space="DRAM"))
        # remote K/V tiles outlive the projection scope -> enter before it;
        # bufs=4 holds every kv head at once so no gather read is ever
        # WAR-blocked at the head of the DMA ring (damming traffic behind it)
        ktpool = top.enter_context(tc.tile_pool(name="ktpool", bufs=4))
        vppool = top.enter_context(tc.tile_pool(name="vppool", bufs=4))

        warmw = resident.tile([128, 128], BF16)
        nc.gpsimd.memset(warmw[:], 0.0)

        # HAM warm-up: the PE clock gate defaults to 1.2 GHz and needs ~3.4us
        # of sustained matmul activity to lift.  These zero-weight matmuls
        # depend on no DMA, so they run while the first input chunks are
        # still in flight and the real matmul stream starts warm.
        with tc.tile_pool(name="warm", bufs=1, space="PSUM") as wp:
            wps = wp.tile([128, 128], F32)
            for _ in range(64):
                nc.tensor.matmul(wps[:], warmw[:], warmw[:],
                                 start=True, stop=True)

        q_sb = [resident.tile([128, NQ], BF16, name=f"q_sb{_h}")
                for _h in range(H)]
        k_loc = resident.tile([128, HKV, NQ], BF16)       # roped local K, d-major
        vp_loc = resident.tile([128, HKV * 4, HD + 1], BF16)  # (hk, local kb)
        nc.gpsimd.memset(vp_loc[:, :, HD:HD + 1], 1.0)
        oT_sb = [resident.tile([128, 4, 128], BF16, name=f"oT{_h}")
                 for _h in range(H)]

        # kv bounce: rows 0..511 = roped K (4 heads x 128 d), cols = local n;
        # rows 512..1023 = V (local n rows), cols = 4 heads x 128 channels
        kv_bounce = dram.tile([2 * NQ, NQ], BF16)
        ag_out = dram.tile([2 * NQ * 4, NQ], BF16)

        # -- projection scope: tensors freed after the Q phase ------------------
        proj_scope = ExitStack()
        proj = proj_scope.enter_context(tc.tile_pool(name="proj", bufs=1))
        tmp_pool = proj_scope.enter_context(tc.tile_pool(name="ropetmp", bufs=3))
        cos_sb = proj.tile([HD, NQ], F32)
        sin_sb = proj.tile([HD, NQ], F32)
        xts = proj.tile([128, DC, NQ], BF16)
        xT_r = xT_d.rearrange("(dc p) n -> p dc n", p=128)
        wq_sb = proj.tile([128, H, DC * 128], BF16)

        # ---------------- KV projection + RoPE(K) + bounce-out ----------------
        with ExitStack() as ph:
            wkpool = ph.enter_context(tc.tile_pool(name="wkpool", bufs=1))
            wvpool = ph.enter_context(tc.tile_pool(name="wvpool", bufs=1))
            kvsb = ph.enter_context(tc.tile_pool(name="kvsb", bufs=3))
            # PSUM entry order fixes bank addresses: kvps 0-3, vsp 4,
            # qps 5-7.  The attention pools then overlap as stA 0-2 (K
            # banks, drained early), ops 3-4 (K/V banks), stB 5-7 (the qps
            # banks, whose pool-coarse WAR on the last Q RoPE reads lands
            # on the SECOND score chunk, ~1.6us late by construction).
            kvps_pool = proj_scope.enter_context(
                tc.tile_pool(name="kvps", bufs=4, space="PSUM"))
            vsp_pool = proj_scope.enter_context(
                tc.tile_pool(name="vsp", bufs=1, space="PSUM"))
            qps_pool = proj_scope.enter_context(
                tc.tile_pool(name="qps", bufs=1, space="PSUM"))

            wk_sb = wkpool.tile([128, HKV, DC, 128], BF16)
            wv_sb = wvpool.tile([128, DC, HKV * HD], BF16)
            wq_r = wq_d.rearrange("h p dc c -> p h (dc c)")
            wk_r = wk_d.rearrange("hk p dc c -> p hk (dc c)")

            # DMA issue order == compute consumption order (the ring drains
            # in order).  Fine chunks up front so the first K matmul needs
            # only ~2.5MB; wq groups 0-1 ride the early stream (Q proj
            # starts the moment the KV phase drains); wq groups 2-3 are
            # emitted after the bounce writes, excluded from the AllGather's
            # position-based readiness gate, and stream inside its trigger
            # delay, clear of the AG data phase.
            nc.sync.dma_start(wk_sb[:, 0:1, :, :], wk_r[:, 0:1, :])
            nc.sync.dma_start(xts[:, 0:4, :], xT_r[:, 0:4, :])
            nc.sync.dma_start(xts[:, 4:8, :], xT_r[:, 4:8, :])
            nc.sync.dma_start(wk_sb[:, 1:2, :, :], wk_r[:, 1:2, :])
            nc.sync.dma_start(xts[:, 8:12, :], xT_r[:, 8:12, :])
            nc.sync.dma_start(wk_sb[:, 2:3, :, :], wk_r[:, 2:3, :])
            nc.sync.dma_start(xts[:, 12:16, :], xT_r[:, 12:16, :])
            nc.sync.dma_start(wk_sb[:, 3:4, :, :], wk_r[:, 3:4, :])
            nc.sync.dma_start(wv_sb[:], wv_d.rearrange("d p c -> p d c"))
            nc.sync.dma_start(cos_sb[:], cosT_d)
            nc.sync.dma_start(sin_sb[:], sinT_d)
            nc.sync.dma_start(wq_sb[:, 0:4, :], wq_r[:, 0:4, :])
            nc.sync.dma_start(wq_sb[:, 4:8, :], wq_r[:, 4:8, :])

            for hk in range(HKV):
                kps = kvps_pool.tile([128, NQ], F32, name="kvps_t")
                for dc in range(DC):
                    nc.tensor.matmul(kps[:], wk_sb[:, hk, dc, :], xts[:, dc, :],
                                     start=(dc == 0), stop=(dc == DC - 1))
                _rope(nc, tmp_pool, k_loc[:, hk, :], kps, cos_sb, sin_sb)
                # scalar-ring DMA: jumps ahead of bulk traffic on the sync
                # ring so the AllGather can start as soon as K/V are ready
                nc.scalar.dma_start(kv_bounce[hk * 128:(hk + 1) * 128, :],
                                    k_loc[:, hk, :])

            # n4-outer with the first tile on the spare PSUM bank: the first
            # V chain starts with no WAR at all, and by the time chain n4=1
            # wants a kvps bank the K RoPE reads (pool-coarse WAR) have
            # drained behind chain n4=0's matmuls.
            vps_tiles = [vsp_pool.tile([128, HKV * HD], F32, name="vsp_t")]
            vps_tiles += [kvps_pool.tile([128, HKV * HD], F32, name="kvps_t")
                          for i in range(3)]
            for n4 in range(4):
                for dc in range(DC):
                    nc.tensor.matmul(
                        vps_tiles[n4][:],
                        xts[:, dc, n4 * 128:(n4 + 1) * 128],
                        wv_sb[:, dc, :],
                        start=(dc == 0), stop=(dc == DC - 1))
            for n4 in range(4):
                v_sb = kvsb.tile([128, HKV * HD], BF16, name="v_sb")
                nc.vector.tensor_copy(v_sb[:], vps_tiles[n4][:])
                nc.scalar.dma_start(
                    kv_bounce[NQ + n4 * 128:NQ + (n4 + 1) * 128, :], v_sb[:])
                for hk in range(HKV):
                    nc.vector.tensor_copy(
                        vp_loc[:, hk * 4 + n4, 0:HD],
                        vps_tiles[n4][:, hk * HD:(hk + 1) * HD])

            # wq groups 2-3 emitted AFTER the bounce DMAs: excluded from the
            # collective's readiness gate, streamed during its trigger delay
            for q4 in range(2, 4):
                nc.sync.dma_start(wq_sb[:, q4 * 4:(q4 + 1) * 4, :],
                                  wq_r[:, q4 * 4:(q4 + 1) * 4, :])

        # ---------------- AllGather K,V within the batch group ----------------
        # Single fused collective (split collectives serialize on the CC core).
        nc.gpsimd.collective_compute(
            "AllGather", mybir.AluOpType.bypass,
            replica_groups=[[0, 1, 2, 3], [4, 5, 6, 7]],
            ins=[kv_bounce.opt()],
            outs=[ag_out.opt()],
        )

        # All four kv-heads' gather reads are emitted NOW so the DMAs fire
        # the moment the collective lands (during Q projection) instead of
        # at attention start.
        pid = nc.sync.partition_id()
        slots = [(pid + i) % 4 for i in (1, 2, 3)]
        hk_tiles = {}

        def load_kv_tiles(hk):
            ktr = ktpool.tile([128, 3, NQ], BF16, name="ktr")
            for i, slot in enumerate(slots):
                nc.sync.dma_start(
                    ktr[:, i, :],
                    ag_out[bass.ds(slot * 2 * NQ + hk * 128, 128), :])
            vpr = vppool.tile([128, 12, HD + 1], BF16, name="vpr")
            nc.gpsimd.memset(vpr[:, :, HD:HD + 1], 1.0)
            for i, slot in enumerate(slots):
                src = ag_out[bass.ds(slot * 2 * NQ + NQ, NQ),
                             hk * HD:(hk + 1) * HD]
                nc.sync.dma_start(
                    vpr[:, i * 4:(i + 1) * 4, 0:HD],
                    src.rearrange("(kbl p) c -> p kbl c", p=128))
            hk_tiles[hk] = (ktr, vpr)

        for _hk in range(HKV):
            load_kv_tiles(_hk)

        # ---------------- Q projection + RoPE ---------------------------------
        # Manual qps rotation (15-h)%3: bank 2 (first st bank the attention
        # scores need) is last touched by h13's RoPE, banks 0-1 (the ops
        # banks, needed a few us later) by h15/h14 -> no attention-start WAR.
        qts = [qps_pool.tile([128, NQ], F32, name=f"qps_t{i}")
               for i in range(3)]
        for h in range(H):
            qps = qts[(15 - h) % 3]
            for dc in range(DC):
                nc.tensor.matmul(qps[:],
                                 wq_sb[:, h, dc * 128:(dc + 1) * 128],
                                 xts[:, dc, :],
                                 start=(dc == 0), stop=(dc == DC - 1))
            _rope(nc, tmp_pool, q_sb[h][:], qps, cos_sb, sin_sb)
        proj_scope.close()

        # Wo lands in the space freed by the projection tensors; its DMAs are
        # emitted inside the attention loop (per kv head) so they queue on
        # the ring behind the AG-gated gather reads and never contend with
        # the collective.
        post = top.enter_context(tc.tile_pool(name="post", bufs=1))
        wo_sb = post.tile([128, H, D], BF16)

        # ---------------- Attention (flat pipelined stream) -------------------
        # One uniform stream of (head, key-block) pairs in triples: each exp
        # op spans head boundaries, so ScalarE (the attention-phase floor)
        # runs continuously and head flushes never stall the PE.
        # PV accumulates 16 blocks per head into two packed 2-slot PSUM tiles
        # (start=True only opens the first slot's chain: it clears has_written
        # for the whole bank, so the second chain opens with start=False on
        # freshly-cleared bits).  Each head's PSUM banks are released by a
        # short raw DVE copy; reciprocal + normalize + the [128,128] XBAR
        # transposes run afterwards, off the inter-head chain.
        # The stream runs in TWO passes to absorb peer-core launch skew (the
        # AllGather's data phase ends only after the slowest peer in the
        # replica group triggers, which can be 30-45us after this core):
        # pass 1 covers every head's LOCAL key blocks (kb 0-3, no collective
        # dependency, ~37us of PE work), spilling each head's partial
        # numerator/denominator to SBUF; pass 2 streams the remote blocks
        # and merges the partials during normalization.
        with ExitStack() as ph:
            ptpool = ph.enter_context(tc.tile_pool(name="ptpool", bufs=5))
            npool = ph.enter_context(tc.tile_pool(name="npool", bufs=4))
            orpool = ph.enter_context(tc.tile_pool(name="orpool", bufs=3))
            opool = ph.enter_context(tc.tile_pool(name="opool", bufs=6))
            lppool = ph.enter_context(tc.tile_pool(name="lppool", bufs=1))
            # all 16 heads' local partials live until pass 2 -> one static tile
            lpart_sb = lppool.tile([128, H, 4, HD + 1], BF16)
            stA_pool = ph.enter_context(tc.tile_pool(name="stA", bufs=1, space="PSUM"))
            ops_pool = ph.enter_context(tc.tile_pool(name="opsp", bufs=1, space="PSUM"))
            stB_pool = ph.enter_context(tc.tile_pool(name="stB", bufs=1, space="PSUM"))
            stt = [stA_pool.tile([128, 3, NQ], F32, name="stA0"),
                   stB_pool.tile([128, 3, NQ], F32, name="stB0")]

            heads = [(hk, g * HKV + hk) for hk in range(HKV) for g in range(G)]

            def kchunk(hk, kb):
                if kb < 4:
                    return k_loc[:, hk, kb * 128:(kb + 1) * 128]
                rb = kb - 4
                return hk_tiles[hk][0][:, rb // 4, (rb % 4) * 128:(rb % 4 + 1) * 128]

            def vchunk(hk, kb):
                if kb < 4:
                    return vp_loc[:, hk * 4 + kb, :]
                return hk_tiles[hk][1][:, kb - 4, :]

            ops_of = {}

            def emit_local_done(hi):
                # spill the local partial (numerator + ones-col denominator)
                # to SBUF, freeing the PSUM banks for the next head
                ops = ops_of.pop(hi)
                for i in range(2):
                    nc.vector.tensor_copy(
                        lpart_sb[:, hi, 2 * i:2 * i + 2, :], ops[i][:])

            def emit_norm(hi):
                _, h = heads[hi]
                ops = ops_of.pop(hi)
                o_part = opool.tile([128, 4, HD], BF16, name="o_part")
                for i in range(2):
                    mrg = orpool.tile([128, 2, HD + 1], F32, name=f"mrg{i}")
                    nc.vector.tensor_tensor(
                        mrg[:], ops[i][:],
                        lpart_sb[:, hi, 2 * i:2 * i + 2, :], ADD)
                    rin = npool.tile([128, 2, 1], F32, name=f"rin{i}")
                    nc.vector.reciprocal(rin[:], mrg[:, :, HD:HD + 1])
                    for j in range(2):
                        qc = i * 2 + j
                        nc.vector.tensor_scalar_mul(
                            o_part[:, qc, :],
                            mrg[:, j, 0:HD], rin[:, j, :])
                nc.sync.dma_start_transpose(oT_sb[h][:], o_part[:])

            def emit_one_pv(item, opener, closer):
                pvt, j, hi, kb = item
                hk, _ = heads[hi]
                for qc in range(4):
                    nc.tensor.matmul(
                        ops_of[hi][qc // 2][:, qc % 2, :],
                        pvt[:, j, qc * 128:(qc + 1) * 128],
                        vchunk(hk, kb),
                        start=(kb == opener and qc % 2 == 0),
                        stop=(kb == closer))
                if kb == closer:
                    if closer == 3:
                        emit_local_done(hi)
                    else:
                        emit_norm(hi)

            from collections import deque
            PV_LAG = 6
            st_ctr = [0]

            def run_stream(units, opener, closer, first_pass):
                pvq = deque()
                chunks = [tuple(units[i:i + 3])
                          for i in range(0, len(units), 3)]
                for chunk in chunks:
                    for hi, kb in chunk:
                        if kb == opener:
                            ops_of[hi] = [
                                ops_pool.tile([128, 2, HD + 1], F32,
                                              name=f"ops{i}")
                                for i in range(2)]
                            hk = heads[hi][0]
                            if not first_pass and hi % G == 0:
                                # wo chunks emitted in the REMOTE pass: they
                                # queue behind the gather reads but ahead of
                                # only a few oT transposes at a time, so the
                                # o_part recycle never dams behind 8MB of wo
                                nc.sync.dma_start(
                                    wo_sb[:, hk * 4:(hk + 1) * 4, :],
                                    wo_d[hk * 4:(hk + 1) * 4]
                                    .rearrange("h p c -> p h c"))
                    st = stt[st_ctr[0] % 2]
                    st_ctr[0] += 1
                    for j, (hi, kb) in enumerate(chunk):
                        hk, h = heads[hi]
                        nc.tensor.matmul(st[:, j, :], kchunk(hk, kb),
                                         q_sb[h][:], start=True, stop=True)
                        if len(pvq) > PV_LAG:
                            emit_one_pv(pvq.popleft(), opener, closer)
                    pt = ptpool.tile([128, 3, NQ], BF16, name="pt_t")
                    nj = len(chunk)
                    nc.scalar.activation(pt[:, 0:nj, :], st[:, 0:nj, :],
                                         EXP, scale=SCALE)
                    for j, (hi, kb) in enumerate(chunk):
                        pvq.append((pt, j, hi, kb))
                while pvq:
                    emit_one_pv(pvq.popleft(), opener, closer)

            run_stream([(hi, kb) for hi in range(len(heads))
                        for kb in range(4)], 0, 3, True)
            run_stream([(hi, kb) for hi in range(len(heads))
                        for kb in range(4, KB)], 4, KB - 1, False)

        # ---------------- Output projection (weights already resident) --------
        with ExitStack() as ph:
            outsb = ph.enter_context(tc.tile_pool(name="outsb", bufs=4))
            outps = ph.enter_context(tc.tile_pool(name="outps", bufs=3, space="PSUM"))
            for dcol in range(4):
                for qc in range(4):
                    outp = outps.tile([128, 512], F32, name="outp")
                    for h in range(H):
                        nc.tensor.matmul(
                            outp[:], oT_sb[h][:, qc, :],
                            wo_sb[:, h, dcol * 512:(dcol + 1) * 512],
                            start=(h == 0), stop=(h == H - 1))
                    osb = outsb.tile([128, 512], BF16, name="osb")
                    nc.vector.tensor_copy(osb[:], outp[:])
                    nc.sync.dma_start(
                        out_d[qc * 128:(qc + 1) * 128,
                              dcol * 512:(dcol + 1) * 512], osb[:])

    nc.compile()
    return nc


def _prep_inputs(x, cos, sin, Wq, Wkv, Wo):
    bf = ml_dtypes.bfloat16
    wq_prep = np.ascontiguousarray(
        Wq.reshape(DC, 128, H, HD).transpose(2, 1, 0, 3)).astype(bf)
    wk_prep = np.ascontiguousarray(
        Wkv[:, :HKV * HD].reshape(DC, 128, HKV, HD).transpose(2, 1, 0, 3)).astype(bf)
    wv_prep = np.ascontiguousarray(
        Wkv[:, HKV * HD:].reshape(DC, 128, HKV * HD)).astype(bf)
    wo_prep = np.ascontiguousarray(Wo.reshape(H, HD, D)).astype(bf)
    c64 = cos[0, :, 0, :].T.astype(np.float32)   # [64, N]
    s64 = sin[0, :, 0, :].T.astype(np.float32)
    cosT = np.ascontiguousarray(np.concatenate([c64, c64], axis=0))   # [128, N]
    sinT = np.ascontiguousarray(np.concatenate([s64, -s64], axis=0))

    in_maps = []
    for c in range(NCORES):
        b, j = divmod(c, 4)
        rows = slice(j * NQ, (j + 1) * NQ)
        xT = np.ascontiguousarray(x[b].T[:, rows]).astype(bf)
        in_maps.append({
            "xT": xT,
            "cosT": np.ascontiguousarray(cosT[:, rows]),
            "sinT": np.ascontiguousarray(sinT[:, rows]),
            "wq": wq_prep, "wk": wk_prep, "wv": wv_prep, "wo": wo_prep,
        })
    return in_maps


def kernel(x, cos, sin, attn_mask, Wq, Wkv, Wo, bo):
    x = np.asarray(x, np.float32)
    cos = np.asarray(cos, np.float32)
    sin = np.asarray(sin, np.float32)
    Wq = np.asarray(Wq, np.float32)
    Wkv = np.asarray(Wkv, np.float32)
    Wo = np.asarray(Wo, np.float32)
    bo = np.asarray(bo, np.float32)

    if "nc" not in _cache:
        _cache["nc"] = _build()
    nc = _cache["nc"]

    in_maps = _prep_inputs(x, cos, sin, Wq, Wkv, Wo)
    res = run_bass_kernel_spmd(nc, in_maps, list(range(NCORES)))
    out = np.empty((B, N, D), np.float32)
    for c in range(NCORES):
        b, j = divmod(c, 4)
        out[b, j * NQ:(j + 1) * NQ, :] = np.asarray(
            res.results[c]["out"], dtype=np.float32)
    out += bo[None, None, :]
    return out
